# revision 7
# baseline (speedup 1.0000x reference)
"""GGNN (gated graph NN) message-passing kernel for 8 Trainium2 NeuronCores.

Sharding: edge-type sharding. Core c owns edge-type block c of the adjacency
matrix (columns c*N..(c+1)*N of the [N, 2E*N] adjacency, pre-transposed on the
host) plus the node shard c for the GRU update.

Per step, on core c:
  stage1: t_c = h @ W_prop[c]                      [N, D]   (h^T streamed as lhsT)
  stage2: partial_a_c = A_cT.T @ t_c               [N, D]   (A_cT resident bf16)
  RS:     a_shard = ReduceScatter_add(partial_a)   [N/8, D]
  GRU:    h_shard' = GRU(a_shard, h_shard)         (transposed layout, fp32r mm)
  AG:     h^T' = AllGather(h_shard'^T)             (fp32r)

Numerics: matmuls in float32r (fp32 with 12-bit mantissa, full PE rate at
free-dim>=256); adjacency in bf16 (exact for 0/1) upconverted to fp32r on the
fly; accumulation fp32 in PSUM; elementwise GRU update in fp32.
"""
import sys
if "/opt/trn_rl_repo" not in sys.path:
    sys.path.insert(0, "/opt/trn_rl_repo")

import numpy as np
import ml_dtypes

NC_CORES = 8
N = 2048          # nodes
D = 512           # state dim
ANN = 256         # annotation dim
STEPS = 5
SH = N // NC_CORES   # 256 nodes per shard
KT = D // 128        # 4
MT = N // 128        # 16


def _q12(x):
    """Round fp32 to 12 explicit mantissa bits (fp32r grid), RNE."""
    mant, ex = np.frexp(np.asarray(x, np.float32).astype(np.float64))
    return (np.round(mant * 4096) / 4096 * np.exp2(ex)).astype(np.float32)


def build():
    import concourse.bacc as bacc
    import concourse.mybir as mybir
    import concourse.tile as tile
    from concourse.masks import make_identity

    dt = mybir.dt
    nc = bacc.Bacc()
    at_p = nc.declare_dram_parameter("at", [N, N], dt.bfloat16, isOutput=False)
    h0t_p = nc.declare_dram_parameter("h0t", [D, N], dt.float32r, isOutput=False)
    h0sr_p = nc.declare_dram_parameter("h0sr", [D, SH], dt.float32r, isOutput=False)
    h0s_p = nc.declare_dram_parameter("h0s", [D, SH], dt.float32, isOutput=False)
    wc_p = nc.declare_dram_parameter("wc", [D, D], dt.float32r, isOutput=False)
    gw_p = nc.declare_dram_parameter("gw", [6, D, D], dt.float32r, isOutput=False)
    bpc_p = nc.declare_dram_parameter("bpc", [1, D], dt.float32, isOutput=False)
    bz_p = nc.declare_dram_parameter("bzc", [D, 1], dt.float32, isOutput=False)
    br_p = nc.declare_dram_parameter("brc", [D, 1], dt.float32, isOutput=False)
    bh_p = nc.declare_dram_parameter("bhc", [D, 1], dt.float32, isOutput=False)
    out_p = nc.declare_dram_parameter("out", [D, SH], dt.float32, isOutput=True)
    RG = [list(range(NC_CORES))]

    from contextlib import ExitStack
    with tile.TileContext(nc) as tc, ExitStack() as stk:
        res = stk.enter_context(tc.tile_pool(name="res", bufs=1))
        p_mm = stk.enter_context(tc.tile_pool(name="pmm", bufs=8, space="PSUM"))
        p_hc = stk.enter_context(tc.tile_pool(name="phc", bufs=2))
        p_t = stk.enter_context(tc.tile_pool(name="pt", bufs=1))
        p_ar = stk.enter_context(tc.tile_pool(name="par", bufs=2))
        p_asb = stk.enter_context(tc.tile_pool(name="pasb", bufs=2))
        p_gw = stk.enter_context(tc.tile_pool(name="pgw", bufs=8))
        p_sm = stk.enter_context(tc.tile_pool(name="psm", bufs=1))
        p_h = stk.enter_context(tc.tile_pool(name="ph", bufs=2))
        dram = stk.enter_context(tc.tile_pool(name="dram", bufs=2, space="DRAM"))

        # ---- setup: constants, weights, adjacency ----
        identity = res.tile([128, 128], dt.float32, tag="identity")
        make_identity(nc, identity[:])
        ones = res.tile([1, 128], dt.float32, tag="ones")
        nc.vector.memset(ones[:], 1.0)
        bpc_t = res.tile([1, D], dt.float32, tag="bpc")
        nc.sync.dma_start(bpc_t[:], bpc_p[:])
        pb = p_mm.tile([128, D], dt.float32, tag="mm")
        nc.tensor.matmul(pb[:], ones[:], bpc_t[:], start=True, stop=True)
        bias_bcast = res.tile([128, D], dt.float32, tag="bias_bcast")
        nc.vector.tensor_copy(bias_bcast[:], pb[:])

        bias_tiles = {}
        for nm, par in (("z", bz_p), ("r", br_p), ("h", bh_p)):
            for f in range(KT):
                bt = res.tile([128, 1], dt.float32, tag=f"b{nm}{f}")
                nc.sync.dma_start(bt[:], par[f * 128:(f + 1) * 128, :])
                bias_tiles[(nm, f)] = bt

        wc_t = []
        for k in range(KT):
            w = res.tile([128, D], dt.float32r, tag=f"wc{k}")
            nc.sync.dma_start(w[:], wc_p[k * 128:(k + 1) * 128, :])
            wc_t.append(w)

        at_t = []
        for m in range(MT):
            a = res.tile([128, N], dt.bfloat16, tag=f"at{m}")
            nc.sync.dma_start(a[:], at_p[m * 128:(m + 1) * 128, :])
            at_t.append(a)

        # step-0 h state
        hsh_prev = []   # h^T shard, fp32r (GRU rhs)
        h32_prev = []   # h^T shard, fp32 (elementwise state)
        for k in range(KT):
            hr = p_h.tile([128, SH], dt.float32r, tag=f"hnr{k}")
            nc.sync.dma_start(hr[:], h0sr_p[k * 128:(k + 1) * 128, :])
            hsh_prev.append(hr)
            h3 = p_h.tile([128, SH], dt.float32, tag=f"h32{k}")
            nc.sync.dma_start(h3[:], h0s_p[k * 128:(k + 1) * 128, :])
            h32_prev.append(h3)

        ag_out_prev = None

        for s in range(STEPS):
            # ---- stage 1: t = h @ W_c  (+ b_c via broadcast add on cast) ----
            t_tiles = []
            for m in range(MT):
                hc = p_hc.tile([128, 4 * 128], dt.float32r, tag="hc")
                if s == 0:
                    src = h0t_p[:, m * 128:(m + 1) * 128]
                else:
                    cp = m // 2
                    cb = (m % 2) * 128
                    src = ag_out_prev[512 * cp:512 * (cp + 1), cb:cb + 128]
                nc.sync.dma_start(hc[:], src.rearrange("(k p) j -> p k j", p=128))
                pt = p_mm.tile([128, D], dt.float32, tag="mm")
                for k in range(KT):
                    nc.tensor.matmul(pt[:], hc[:, k * 128:(k + 1) * 128], wc_t[k][:],
                                     start=(k == 0), stop=(k == KT - 1))
                tm = p_t.tile([128, D], dt.float32r, tag=f"t{m}")
                nc.vector.tensor_add(tm[:], pt[:], bias_bcast[:])
                t_tiles.append(tm)

            # ---- stage 2: partial_a = A_cT.T @ t ----
            rs_in = dram.tile([N, D], dt.float32, tag="rs_in")
            for grp in range(2):
                pas = [p_mm.tile([128, D], dt.float32, tag="mm", name=f"pa{grp}_{i}")
                       for i in range(8)]
                for m in range(MT):
                    ar = p_ar.tile([128, 1024], dt.float32r, tag="ar")
                    nc.vector.tensor_copy(ar[:], at_t[m][:, grp * 1024:(grp + 1) * 1024])
                    for i in range(8):
                        nc.tensor.matmul(pas[i][:], ar[:, i * 128:(i + 1) * 128],
                                         t_tiles[m][:],
                                         start=(m == 0), stop=(m == MT - 1))
                for i in range(8):
                    n = grp * 8 + i
                    asb = p_asb.tile([128, D], dt.float32, tag="asb")
                    nc.vector.tensor_copy(asb[:], pas[i][:])
                    nc.sync.dma_start(rs_in[n * 128:(n + 1) * 128, :], asb[:])

            # ---- ReduceScatter: each core gets its 256-node shard of a ----
            rs_out = dram.tile([SH, D], dt.float32, tag="rs_out")
            nc.gpsimd.collective_compute(
                "ReduceScatter", mybir.AluOpType.add, replica_groups=RG,
                ins=[rs_in[:]], outs=[rs_out[:]])

            # ---- transpose a_shard -> aT [D, SH] fp32r ----
            an_tiles = []
            for r2 in range(2):
                an = p_sm.tile([128, D], dt.float32, tag=f"an{r2}")
                nc.sync.dma_start(an[:], rs_out[r2 * 128:(r2 + 1) * 128, :])
                an_tiles.append(an)
            aT = []
            for kb in range(KT):
                a_kb = p_sm.tile([128, SH], dt.float32r, tag=f"aT{kb}")
                for r2 in range(2):
                    ptr = p_mm.tile([128, 128], dt.float32, tag="mm")
                    nc.tensor.transpose(ptr[:], an_tiles[r2][:, kb * 128:(kb + 1) * 128],
                                        identity[:])
                    nc.vector.tensor_copy(a_kb[:, r2 * 128:(r2 + 1) * 128], ptr[:])
                aT.append(a_kb)

            # ---- GRU gates (transposed layout [D, SH]) ----
            def gate_mm(widx, uidx, rhs_u, func, bias_nm, out_dtype=dt.float32):
                Wt = []
                for k in range(KT):
                    w = p_gw.tile([128, D], dt.float32r, tag="gw")
                    nc.sync.dma_start(w[:], gw_p[widx, k * 128:(k + 1) * 128, :])
                    Wt.append(w)
                Ut = []
                for k in range(KT):
                    u = p_gw.tile([128, D], dt.float32r, tag="gw")
                    nc.sync.dma_start(u[:], gw_p[uidx, k * 128:(k + 1) * 128, :])
                    Ut.append(u)
                outs = []
                for f in range(KT):
                    pg = p_mm.tile([128, SH], dt.float32, tag="mm")
                    for k in range(KT):
                        nc.tensor.matmul(pg[:], Wt[k][:, f * 128:(f + 1) * 128],
                                         aT[k][:], start=(k == 0), stop=False)
                    for k in range(KT):
                        nc.tensor.matmul(pg[:], Ut[k][:, f * 128:(f + 1) * 128],
                                         rhs_u[k][:], start=False, stop=(k == KT - 1))
                    og = p_sm.tile([128, SH], out_dtype, tag=f"g{bias_nm}{f}")
                    nc.scalar.activation(og[:], pg[:], func,
                                         bias=bias_tiles[(bias_nm, f)][:])
                    outs.append(og)
                return outs

            import concourse.mybir as _mb
            z_t = gate_mm(0, 1, hsh_prev, _mb.ActivationFunctionType.Sigmoid, "z")
            r_t = gate_mm(2, 3, hsh_prev, _mb.ActivationFunctionType.Sigmoid, "r")
            rh = []
            for k in range(KT):
                rhk = p_sm.tile([128, SH], dt.float32r, tag=f"rh{k}")
                nc.vector.tensor_mul(rhk[:], r_t[k][:], h32_prev[k][:])
                rh.append(rhk)
            ht_t = gate_mm(4, 5, rh, _mb.ActivationFunctionType.Tanh, "h")

            # ---- h' = h + z * (ht - h) ----
            hsh_new, h32_new = [], []
            last = (s == STEPS - 1)
            if not last:
                ag_in = dram.tile([D, SH], dt.float32r, tag="ag_in")
            for k in range(KT):
                s1 = p_sm.tile([128, SH], dt.float32, tag="gsA")
                nc.vector.tensor_sub(s1[:], ht_t[k][:], h32_prev[k][:])
                s2 = p_sm.tile([128, SH], dt.float32, tag="gsB")
                nc.vector.tensor_mul(s2[:], z_t[k][:], s1[:])
                h3 = p_h.tile([128, SH], dt.float32, tag=f"h32{k}")
                nc.vector.tensor_add(h3[:], h32_prev[k][:], s2[:])
                h32_new.append(h3)
                if last:
                    nc.sync.dma_start(out_p[k * 128:(k + 1) * 128, :], h3[:])
                else:
                    hr = p_h.tile([128, SH], dt.float32r, tag=f"hnr{k}")
                    nc.vector.tensor_copy(hr[:], h3[:])
                    hsh_new.append(hr)
                    nc.sync.dma_start(ag_in[k * 128:(k + 1) * 128, :], hr[:])

            if not last:
                ag_out = dram.tile([NC_CORES * D, SH], dt.float32r, tag="ag_out",
                                   addr_space="Shared")
                nc.gpsimd.collective_compute(
                    "AllGather", mybir.AluOpType.bypass, replica_groups=RG,
                    ins=[ag_in[:]], outs=[ag_out[:]])
                ag_out_prev = ag_out
                hsh_prev, h32_prev = hsh_new, h32_new

    nc.finalize()
    return nc


_BUILT = None
TRACE = False
LAST_RESULT = None


def _get_built():
    global _BUILT
    if _BUILT is None:
        _BUILT = build()
    return _BUILT


def prepare_in_maps(adjacency, annotations, W_prop, b_prop, Wz, Uz, bz,
                    Wr, Ur, br, Wh, Uh, bh):
    A = np.asarray(adjacency, np.float32)
    ann = np.asarray(annotations, np.float32)
    W_prop = np.asarray(W_prop, np.float32)
    b_prop = np.asarray(b_prop, np.float32)
    gw_all = _q12(np.stack([np.asarray(x, np.float32)
                            for x in (Wz, Uz, Wr, Ur, Wh, Uh)]))
    bz = np.asarray(bz, np.float32).reshape(D, 1)
    br = np.asarray(br, np.float32).reshape(D, 1)
    bh = np.asarray(bh, np.float32).reshape(D, 1)

    h0 = np.zeros((N, D), np.float32)
    h0[:, :ann.shape[1]] = ann
    h0t = np.ascontiguousarray(h0.T)           # [D, N] fp32
    h0t_r = _q12(h0t)
    A_T = np.ascontiguousarray(A.T)            # [2E*N, N]

    in_maps = []
    for c in range(NC_CORES):
        in_maps.append({
            "at": np.ascontiguousarray(
                A_T[c * N:(c + 1) * N, :]).astype(ml_dtypes.bfloat16),
            "h0t": h0t_r,
            "h0sr": np.ascontiguousarray(h0t_r[:, c * SH:(c + 1) * SH]),
            "h0s": np.ascontiguousarray(h0t[:, c * SH:(c + 1) * SH]),
            "wc": _q12(W_prop[c]),
            "gw": gw_all,
            "bpc": np.ascontiguousarray(b_prop[c].reshape(1, D)),
            "bzc": bz, "brc": br, "bhc": bh,
        })

    return in_maps


def kernel(**inputs):
    from concourse.bass_utils import run_bass_kernel_spmd

    in_maps = prepare_in_maps(
        **{k: inputs[k] for k in ("adjacency", "annotations", "W_prop", "b_prop",
                                  "Wz", "Uz", "bz", "Wr", "Ur", "br",
                                  "Wh", "Uh", "bh")})
    nc = _get_built()
    res = run_bass_kernel_spmd(nc, in_maps, list(range(NC_CORES)), trace=TRACE)
    global LAST_RESULT
    LAST_RESULT = res
    h = np.concatenate([res.results[c]["out"].T for c in range(NC_CORES)], axis=0)
    return np.ascontiguousarray(h, dtype=np.float32)


# revision 11
# speedup vs baseline: 106.4158x; 106.4158x over previous
"""GGNN (gated graph NN) message-passing kernel for 8 Trainium2 NeuronCores.

Sharding: edge-type sharding. Core c owns edge-type block c of the adjacency
matrix (columns c*N..(c+1)*N of the [N, 2E*N] adjacency, pre-transposed on the
host) plus the node shard c for the GRU update.

Per step, on core c:
  stage1: t_c = h @ W_prop[c]                      [N, D]   (h^T streamed as lhsT)
  stage2: partial_a_c = A_cT.T @ t_c               [N, D]   (A_cT resident uint8)
  RS:     a_shard = ReduceScatter_add(partial_a)   [N/8, D] (split in 2 halves so
          the first RS overlaps the second half of stage2)
  GRU:    h_shard' = GRU(a_shard, h_shard)         (transposed layout, fp32r mm)
  AG:     h^T' = AllGather(h_shard'^T)             (fp32r)

Each core's node shard is blocks {128c..128c+127, 1024+128c..1024+128c+127}
(the blocks the two half-ReduceScatters deliver to rank c).

Numerics: matmuls in float32r (fp32 with 12-bit mantissa, full PE rate at
free-dim>=256); adjacency stored as uint8 (exact for 0/1) upconverted to fp32r
on DVE; accumulation fp32 in PSUM; elementwise GRU update in fp32.
"""
import sys
if "/opt/trn_rl_repo" not in sys.path:
    sys.path.insert(0, "/opt/trn_rl_repo")

import numpy as np
import ml_dtypes

NC_CORES = 8
N = 2048          # nodes
D = 512           # state dim
ANN = 256         # annotation dim
STEPS = 5
SH = N // NC_CORES   # 256 nodes per shard
KT = D // 128        # 4
MT = N // 128        # 16


def _q12(x):
    """Round fp32 to 12 explicit mantissa bits (fp32r grid), RNE."""
    mant, ex = np.frexp(np.asarray(x, np.float32).astype(np.float64))
    return (np.round(mant * 4096) / 4096 * np.exp2(ex)).astype(np.float32)


def build(repeats=1, ablate=()):
    import concourse.bacc as bacc
    import concourse.mybir as mybir
    import concourse.tile as tile
    from concourse.masks import make_identity

    dt = mybir.dt
    nc = bacc.Bacc()
    at_p = nc.declare_dram_parameter("at", [N, N], dt.uint8, isOutput=False)
    h0t_p = nc.declare_dram_parameter("h0t", [NC_CORES * D, SH], dt.float32r,
                                      isOutput=False)
    h0sr_p = nc.declare_dram_parameter("h0sr", [D, SH], dt.float32r, isOutput=False)
    h0s_p = nc.declare_dram_parameter("h0s", [D, SH], dt.float32, isOutput=False)
    wc_p = nc.declare_dram_parameter("wc", [D, D], dt.float32r, isOutput=False)
    gw_p = nc.declare_dram_parameter("gw", [6, D, D], dt.float32r, isOutput=False)
    bpc_p = nc.declare_dram_parameter("bpc", [1, D], dt.float32, isOutput=False)
    bz_p = nc.declare_dram_parameter("bzc", [D, 1], dt.float32, isOutput=False)
    br_p = nc.declare_dram_parameter("brc", [D, 1], dt.float32, isOutput=False)
    bh_p = nc.declare_dram_parameter("bhc", [D, 1], dt.float32, isOutput=False)
    out_p = nc.declare_dram_parameter("out", [D, SH], dt.float32, isOutput=True)
    RG = [list(range(NC_CORES))]

    from contextlib import ExitStack
    with tile.TileContext(nc) as tc, ExitStack() as stk:
        res = stk.enter_context(tc.tile_pool(name="res", bufs=1))
        p_mm = stk.enter_context(tc.tile_pool(name="pmm", bufs=8, space="PSUM"))
        p_hc = stk.enter_context(tc.tile_pool(name="phc", bufs=6))
        p_t = stk.enter_context(tc.tile_pool(name="pt", bufs=1))
        p_ar = stk.enter_context(tc.tile_pool(name="par", bufs=3))
        p_asb = stk.enter_context(tc.tile_pool(name="pasb", bufs=2))
        p_sm = stk.enter_context(tc.tile_pool(name="psm", bufs=1))
        p_h = stk.enter_context(tc.tile_pool(name="ph", bufs=2))
        dram = stk.enter_context(tc.tile_pool(name="dram", bufs=2, space="DRAM"))

        # ---- setup: constants, weights, adjacency ----
        identity = res.tile([128, 128], dt.float32, tag="identity")
        make_identity(nc, identity[:])
        ones = res.tile([1, 128], dt.float32, tag="ones")
        nc.vector.memset(ones[:], 1.0)
        bpc_t = res.tile([1, D], dt.float32, tag="bpc")
        nc.sync.dma_start(bpc_t[:], bpc_p[:])
        pb = p_mm.tile([128, D], dt.float32, tag="mm")
        nc.tensor.matmul(pb[:], ones[:], bpc_t[:], start=True, stop=True)
        bias_bcast = res.tile([128, D], dt.float32, tag="bias_bcast")
        nc.vector.tensor_copy(bias_bcast[:], pb[:])

        bias_tiles = {}
        for nm, par in (("z", bz_p), ("r", br_p), ("h", bh_p)):
            for f in range(KT):
                bt = res.tile([128, 1], dt.float32, tag=f"b{nm}{f}")
                nc.sync.dma_start(bt[:], par[f * 128:(f + 1) * 128, :])
                bias_tiles[(nm, f)] = bt

        wc_t = []
        for k in range(KT):
            w = res.tile([128, D], dt.float32r, tag=f"wc{k}")
            nc.sync.dma_start(w[:], wc_p[k * 128:(k + 1) * 128, :])
            wc_t.append(w)

        at_t = []
        for m in range(MT):
            a = res.tile([128, N], dt.uint8, tag=f"at{m}")
            nc.sync.dma_start(a[:], at_p[m * 128:(m + 1) * 128, :])
            at_t.append(a)

        # resident GRU weights (fp32r), loaded once
        gw_res = []
        for g in range(6):
            w = res.tile([128, KT, D], dt.float32r, tag=f"gwr{g}")
            nc.scalar.dma_start(w[:], gw_p[g].rearrange("(k p) f -> p k f", p=128))
            gw_res.append(w)

        for rep in range(repeats):
          # step-0 h state
          hsh_prev = []   # h^T shard, fp32r (GRU rhs)
          h32_prev = []   # h^T shard, fp32 (elementwise state)
          for k in range(KT):
            hr = p_h.tile([128, SH], dt.float32r, tag=f"hnr{k}")
            nc.sync.dma_start(hr[:], h0sr_p[k * 128:(k + 1) * 128, :])
            hsh_prev.append(hr)
            h3 = p_h.tile([128, SH], dt.float32, tag=f"h32{k}")
            nc.sync.dma_start(h3[:], h0s_p[k * 128:(k + 1) * 128, :])
            h32_prev.append(h3)

          ag_out_prev = None

          for s in range(STEPS):
             # ---- stage 1: t = h @ W_c  (+ b_c via broadcast add on cast) ----
             # shard layout: core cp owns node blocks {128cp, 1024+128cp}
             t_tiles = [None] * MT
             for mp in range(MT // 2):
                 if "s1" not in ablate:
                     hc = p_hc.tile([128, KT, 2, 128], dt.float32r, tag="hc")
                     blk = (h0t_p if s == 0 else ag_out_prev)[512 * mp:512 * (mp + 1), :]
                     nc.sync.dma_start(
                         hc[:], blk.rearrange("(k p) mj -> p k mj", p=128))
                 for mloc in range(2):
                     m = mp + 8 * mloc
                     pt = p_mm.tile([128, D], dt.float32, tag="mm")
                     if "s1" in ablate:
                         nc.tensor.matmul(pt[:], wc_t[0][:, 0:128], wc_t[1][:],
                                          start=True, stop=True)
                     else:
                         for k in range(KT):
                             nc.tensor.matmul(pt[:], hc[:, k, mloc, :], wc_t[k][:],
                                              start=(k == 0), stop=(k == KT - 1))
                     tm = p_t.tile([128, D], dt.float32r, tag=f"t{m}")
                     nc.vector.tensor_add(tm[:], pt[:], bias_bcast[:])
                     t_tiles[m] = tm

             # ---- stage 2: partial_a = A_cT.T @ t; RS per half (overlapped) ----
             rs_in = dram.tile([N, D], dt.float32, tag="rs_in")
             rs_outs = []
             for grp in range(2):
                 pas = [p_mm.tile([128, D], dt.float32, tag="mm", name=f"pa{grp}_{i}")
                        for i in range(8)]
                 if "s2" in ablate:
                     for i in range(8):
                         nc.tensor.matmul(pas[i][:], t_tiles[0][:, 0:128],
                                          t_tiles[1][:], start=True, stop=True)
                 else:
                  for m in range(MT):
                     ar = p_ar.tile([128, 1024], dt.float32r, tag="ar")
                     nc.vector.tensor_copy(ar[:], at_t[m][:, grp * 1024:(grp + 1) * 1024])
                     for i in range(8):
                         nc.tensor.matmul(pas[i][:], ar[:, i * 128:(i + 1) * 128],
                                          t_tiles[m][:],
                                          start=(m == 0), stop=(m == MT - 1))
                 for i in range(8):
                     n = grp * 8 + i
                     asb = p_asb.tile([128, D], dt.float32, tag="asb")
                     nc.scalar.copy(asb[:], pas[i][:])
                     eng = nc.sync if i % 2 == 0 else nc.scalar
                     eng.dma_start(rs_in[n * 128:(n + 1) * 128, :], asb[:])
                 # RS of this half: core c receives node block grp*1024 + 128c
                 rs_out = dram.tile([128, D], dt.float32, tag=f"rs_out{grp}",
                                    name=f"rs_out{grp}")
                 if "cc" in ablate or "rs" in ablate:
                     nc.sync.dma_start(rs_out[:],
                                       rs_in[grp * 1024:grp * 1024 + 128, :])
                 else:
                     nc.gpsimd.collective_compute(
                         "ReduceScatter", mybir.AluOpType.add, replica_groups=RG,
                         ins=[rs_in[grp * 1024:(grp + 1) * 1024, :]], outs=[rs_out[:]])
                 rs_outs.append(rs_out)

             # ---- transpose a_shard -> aT [D, SH] fp32r ----
             # r2=0 chunks (from RS1) transpose while RS2 is still in flight
             an_tiles = []
             for r2 in range(2):
                 an = p_sm.tile([128, D], dt.float32, tag=f"an{r2}")
                 nc.sync.dma_start(an[:], rs_outs[r2][:])
                 an_tiles.append(an)
             aT = []
             for kb in range(KT):
                 a_kb = p_sm.tile([128, SH], dt.float32r, tag=f"aT{kb}")
                 aT.append(a_kb)
             for r2 in range(2):
                 for kb in range(KT):
                     ptr = p_mm.tile([128, 128], dt.float32, tag="mm")
                     nc.tensor.transpose(ptr[:], an_tiles[r2][:, kb * 128:(kb + 1) * 128],
                                         identity[:])
                     nc.vector.tensor_copy(aT[kb][:, r2 * 128:(r2 + 1) * 128], ptr[:])

             # ---- GRU gates (transposed layout [D, SH]) ----
             def gate_mm(widx, uidx, rhs_u, func, bias_nm, out_dtype=dt.float32):
                 Wq, Uq = gw_res[widx], gw_res[uidx]
                 outs = []
                 for f in range(KT):
                     pg = p_mm.tile([128, SH], dt.float32, tag="mm")
                     if "gru" in ablate:
                         nc.tensor.matmul(pg[:], aT[0][:, 0:128], aT[0][:],
                                          start=True, stop=True)
                         nc.tensor.matmul(pg[:], rhs_u[0][:, 0:128], rhs_u[0][:],
                                          start=False, stop=True)
                         k = None
                     else:
                      for k in range(KT):
                         nc.tensor.matmul(pg[:], Uq[:, k, f * 128:(f + 1) * 128],
                                          rhs_u[k][:], start=(k == 0), stop=False)
                      for k in range(KT):
                         nc.tensor.matmul(pg[:], Wq[:, k, f * 128:(f + 1) * 128],
                                          aT[k][:], start=False, stop=(k == KT - 1))
                     og = p_sm.tile([128, SH], out_dtype, tag=f"g{bias_nm}{f}")
                     nc.scalar.activation(og[:], pg[:], func,
                                          bias=bias_tiles[(bias_nm, f)][:])
                     outs.append(og)
                 return outs

             import concourse.mybir as _mb
             z_t = gate_mm(0, 1, hsh_prev, _mb.ActivationFunctionType.Sigmoid, "z")
             r_t = gate_mm(2, 3, hsh_prev, _mb.ActivationFunctionType.Sigmoid, "r")
             rh = []
             for k in range(KT):
                 rhk = p_sm.tile([128, SH], dt.float32r, tag=f"rh{k}")
                 nc.vector.tensor_mul(rhk[:], r_t[k][:], h32_prev[k][:])
                 rh.append(rhk)
             ht_t = gate_mm(4, 5, rh, _mb.ActivationFunctionType.Tanh, "h")

             # ---- h' = h + z * (ht - h) ----
             hsh_new, h32_new = [], []
             last = (s == STEPS - 1)
             if not last:
                 ag_in = dram.tile([D, SH], dt.float32r, tag="ag_in")
             for k in range(KT):
                 s1 = p_sm.tile([128, SH], dt.float32, tag="gsA")
                 nc.vector.tensor_sub(s1[:], ht_t[k][:], h32_prev[k][:])
                 s2 = p_sm.tile([128, SH], dt.float32, tag="gsB")
                 nc.vector.tensor_mul(s2[:], z_t[k][:], s1[:])
                 h3 = p_h.tile([128, SH], dt.float32, tag=f"h32{k}")
                 nc.vector.tensor_add(h3[:], h32_prev[k][:], s2[:])
                 h32_new.append(h3)
                 if last:
                     nc.sync.dma_start(out_p[k * 128:(k + 1) * 128, :], h3[:])
                 else:
                     hr = p_h.tile([128, SH], dt.float32r, tag=f"hnr{k}")
                     nc.vector.tensor_copy(hr[:], h3[:])
                     hsh_new.append(hr)
                     nc.sync.dma_start(ag_in[k * 128:(k + 1) * 128, :], hr[:])

             if not last:
                 ag_out = dram.tile([NC_CORES * D, SH], dt.float32r, tag="ag_out",
                                    addr_space="Shared")
                 if "cc" in ablate or "ag" in ablate:
                     nc.sync.dma_start(ag_out[0:D, :], ag_in[:])
                 else:
                     nc.gpsimd.collective_compute(
                         "AllGather", mybir.AluOpType.bypass, replica_groups=RG,
                         ins=[ag_in[:]], outs=[ag_out[:]])
                 ag_out_prev = ag_out
                 hsh_prev, h32_prev = hsh_new, h32_new

    nc.finalize()
    return nc


_BUILT = None
TRACE = False
LAST_RESULT = None


_BUILT_R = {}


def _get_built(repeats=1, ablate=()):
    global _BUILT
    key = (repeats, tuple(ablate))
    if key != (1, ()):
        if key not in _BUILT_R:
            _BUILT_R[key] = build(repeats, ablate)
        return _BUILT_R[key]
    if _BUILT is None:
        _BUILT = build()
    return _BUILT


def prepare_in_maps(adjacency, annotations, W_prop, b_prop, Wz, Uz, bz,
                    Wr, Ur, br, Wh, Uh, bh):
    A = np.asarray(adjacency, np.float32)
    ann = np.asarray(annotations, np.float32)
    W_prop = np.asarray(W_prop, np.float32)
    b_prop = np.asarray(b_prop, np.float32)
    gw_all = _q12(np.stack([np.asarray(x, np.float32)
                            for x in (Wz, Uz, Wr, Ur, Wh, Uh)]))
    bz = np.asarray(bz, np.float32).reshape(D, 1)
    br = np.asarray(br, np.float32).reshape(D, 1)
    bh = np.asarray(bh, np.float32).reshape(D, 1)

    h0 = np.zeros((N, D), np.float32)
    h0[:, :ann.shape[1]] = ann
    h0t = np.ascontiguousarray(h0.T)           # [D, N] fp32
    h0t_r = _q12(h0t)
    A_T = np.ascontiguousarray(A.T)            # [2E*N, N]

    # shard layout: core c owns node blocks {128c..128c+127, 1024+128c..+127}
    shard_cols = [np.r_[128 * c:128 * c + 128, 1024 + 128 * c:1024 + 128 * c + 128]
                  for c in range(NC_CORES)]
    h0t_ag = np.ascontiguousarray(np.concatenate(
        [h0t_r[:, shard_cols[c]] for c in range(NC_CORES)], axis=0))

    in_maps = []
    for c in range(NC_CORES):
        in_maps.append({
            "at": np.ascontiguousarray(
                A_T[c * N:(c + 1) * N, :]).astype(np.uint8),
            "h0t": h0t_ag,
            "h0sr": np.ascontiguousarray(h0t_r[:, shard_cols[c]]),
            "h0s": np.ascontiguousarray(h0t[:, shard_cols[c]]),
            "wc": _q12(W_prop[c]),
            "gw": gw_all,
            "bpc": np.ascontiguousarray(b_prop[c].reshape(1, D)),
            "bzc": bz, "brc": br, "bhc": bh,
        })

    return in_maps


def kernel(**inputs):
    from concourse.bass_utils import run_bass_kernel_spmd

    in_maps = prepare_in_maps(
        **{k: inputs[k] for k in ("adjacency", "annotations", "W_prop", "b_prop",
                                  "Wz", "Uz", "bz", "Wr", "Ur", "br",
                                  "Wh", "Uh", "bh")})
    nc = _get_built()
    res = run_bass_kernel_spmd(nc, in_maps, list(range(NC_CORES)), trace=TRACE)
    global LAST_RESULT
    LAST_RESULT = res
    h = np.empty((N, D), np.float32)
    for c in range(NC_CORES):
        sh = res.results[c]["out"].T           # [SH, D] rows in shard order
        h[128 * c:128 * c + 128] = sh[:128]
        h[1024 + 128 * c:1024 + 128 * c + 128] = sh[128:]
    return h

